# revision 12
# baseline (speedup 1.0000x reference)
"""ConvGRUBandCell2d fused Trainium2 kernel (8 NeuronCores, batch-parallel).

Reference computation (per pixel (b, f), channels C=512):
  xg = xW @ rmsnorm(x_t; in_w) + xb
  hg = hW @ depthwise_band(rmsnorm(h_prev; hid_w); hmixW, hmixb) + hb
  r = sigmoid(xg_r + hg_r); z = sigmoid(xg_z + hg_z)
  n = tanh(xg_n + r * hg_n)
  h_new = (1 - z) * n + z * h_prev
  out = rmsnorm(h_new + x_t; out_w)

Algebraic refactoring used here (all exact):
  - in_norm_w folds into xW columns; hid_norm_w folds into the depthwise taps;
    hmixb folds into an effective bias bh = hW @ hmixb + hb.
  - The per-pixel rms scale commutes with the channel contraction, so x is
    scaled by inv_x before the matmul instead of materializing the norm.
  - xg_r + hg_r is accumulated directly in PSUM by chaining the xW and hW
    matmuls into one accumulation group (r and z gates need no vector add).

Sharding: data-parallel over batch, 8 batches per core, no collectives.
"""

import numpy as np

B, C, F, K = 64, 512, 1024, 3
N_CORES = 8
BPC = B // N_CORES          # batches per core
TC = C // 128               # channel tiles (4)
M3 = (3 * C) // 128         # gate-row tiles (12)
NCH = F // 512              # 512-pixel chunks per batch (2)
EPS = 1e-6

_CACHE = {}


def _build_program():
    import concourse.bacc as bacc
    import concourse.tile as tile
    from concourse import mybir

    f32 = mybir.dt.float32
    f32r = mybir.dt.float32r
    AF = mybir.ActivationFunctionType
    OP = mybir.AluOpType

    nc = bacc.Bacc("TRN2", target_bir_lowering=False, debug=False,
                   num_devices=N_CORES)

    xd = nc.dram_tensor("x", [BPC, C, F], f32, kind="ExternalInput").ap()
    hd = nc.dram_tensor("h", [BPC, C, F], f32, kind="ExternalInput").ap()
    xWTd = nc.dram_tensor("xWT", [C, 3 * C], f32r, kind="ExternalInput").ap()
    hWTd = nc.dram_tensor("hWT", [C, 3 * C], f32r, kind="ExternalInput").ap()
    w3d = nc.dram_tensor("w3", [C, K], f32, kind="ExternalInput").ap()
    gbd = nc.dram_tensor("gb", [3 * C, 1], f32, kind="ExternalInput").ap()
    bhnd = nc.dram_tensor("bhn", [C, 1], f32, kind="ExternalInput").ap()
    xbnd = nc.dram_tensor("xbn", [C, 1], f32, kind="ExternalInput").ap()
    wond = nc.dram_tensor("won", [C, 1], f32, kind="ExternalInput").ap()
    onesd = nc.dram_tensor("ones_in", [128, 128], f32r,
                           kind="ExternalInput").ap()
    outd = nc.dram_tensor("out", [BPC, C, F], f32, kind="ExternalOutput").ap()

    with tile.TileContext(nc) as tc:
        with (
            tc.tile_pool(name="wp", bufs=1) as wp,
            tc.tile_pool(name="sb", bufs=1) as sb,
            tc.tile_pool(name="pp", bufs=2, space="PSUM") as pp,
        ):
            # ---- resident weights / constants ----
            xw_s, hw_s, w3t = [], [], []
            for k in range(TC):
                xw = wp.tile([128, 3 * C], f32r, tag=f"xw{k}", name=f"xw{k}")
                nc.sync.dma_start(xw[:], xWTd[k * 128:(k + 1) * 128, :])
                xw_s.append(xw)
                hw = wp.tile([128, 3 * C], f32r, tag=f"hw{k}", name=f"hw{k}")
                nc.sync.dma_start(hw[:], hWTd[k * 128:(k + 1) * 128, :])
                hw_s.append(hw)
                w3 = wp.tile([128, K], f32, tag=f"w3{k}", name=f"w3{k}")
                nc.sync.dma_start(w3[:], w3d[k * 128:(k + 1) * 128, :])
                w3t.append(w3)
            ones = wp.tile([128, 128], f32r, tag="ones", name="ones")
            nc.sync.dma_start(ones[:], onesd[:, :])
            epst = wp.tile([128, 1], f32, tag="epst", name="epst")
            nc.vector.memset(epst[:], EPS)
            gbt = wp.tile([128, M3], f32, tag="gbt", name="gbt")
            nc.sync.dma_start(gbt[:], gbd.rearrange("(m p) o -> p (m o)", p=128))
            bhnt = wp.tile([128, TC], f32, tag="bhnt", name="bhnt")
            nc.sync.dma_start(bhnt[:], bhnd.rearrange("(m p) o -> p (m o)", p=128))
            xbnt = wp.tile([128, TC], f32, tag="xbnt", name="xbnt")
            nc.sync.dma_start(xbnt[:], xbnd.rearrange("(m p) o -> p (m o)", p=128))
            wont = wp.tile([128, TC], f32, tag="wont", name="wont")
            nc.sync.dma_start(wont[:], wond.rearrange("(m p) o -> p (m o)", p=128))

            onr = ones[:].bitcast(f32r)

            for b in range(BPC):
                # ---------- h path: load, ssq, inv_h, hs ----------
                ht = []
                for ct in range(TC):
                    t = sb.tile([128, F], f32, tag=f"ht{ct}", name=f"ht{b}_{ct}")
                    nc.sync.dma_start(t[:], hd[b, ct * 128:(ct + 1) * 128, :])
                    ht.append(t)
                hs = []
                for ct in range(TC):
                    t = sb.tile([128, F + 2], f32, tag=f"hs{ct}", name=f"hs{b}_{ct}")
                    nc.scalar.square(t[:, 1:F + 1].bitcast(f32r), ht[ct][:])
                    hs.append(t)
                invh = sb.tile([128, F], f32, tag="invh", name=f"invh{b}")
                for ch in range(NCH):
                    ps = pp.tile([128, 512], f32, tag="nrm", name=f"hps{b}_{ch}")
                    for ct in range(TC):
                        nc.tensor.matmul(
                            ps[:], onr,
                            hs[ct][:, 1 + ch * 512: 1 + ch * 512 + 512].bitcast(f32r),
                            start=(ct == 0), stop=(ct == TC - 1))
                    sq = sb.tile([128, 512], f32, tag="sq", bufs=3, name=f"hsq{b}_{ch}")
                    nc.scalar.activation(sq[:], ps[:], AF.Sqrt,
                                         bias=epst[:, 0:1], scale=1.0 / C)
                    nc.vector.reciprocal_approx_fast(
                        invh[:, ch * 512:(ch + 1) * 512], sq[:])
                for ct in range(TC):
                    nc.vector.memset(hs[ct][:, 0:1], 0.0)
                    nc.vector.memset(hs[ct][:, F + 1:F + 2], 0.0)
                    nc.vector.tensor_mul(hs[ct][:, 1:F + 1].bitcast(f32r),
                                         ht[ct][:], invh[:])

                # ---------- x path ----------
                xt = []
                for ct in range(TC):
                    t = sb.tile([128, F], f32, tag=f"xt{ct}", name=f"xt{b}_{ct}")
                    nc.sync.dma_start(t[:], xd[b, ct * 128:(ct + 1) * 128, :])
                    xt.append(t)
                xs = []
                for ct in range(TC):
                    t = sb.tile([128, F], f32, tag=f"xs{ct}", name=f"xs{b}_{ct}")
                    nc.scalar.square(t[:].bitcast(f32r), xt[ct][:])
                    xs.append(t)
                invx = sb.tile([128, F], f32, tag="invx", name=f"invx{b}")
                for ch in range(NCH):
                    ps = pp.tile([128, 512], f32, tag="nrm", name=f"xps{b}_{ch}")
                    for ct in range(TC):
                        nc.tensor.matmul(
                            ps[:], onr,
                            xs[ct][:, ch * 512:(ch + 1) * 512].bitcast(f32r),
                            start=(ct == 0), stop=(ct == TC - 1))
                    sq = sb.tile([128, 512], f32, tag="sq", bufs=3, name=f"xsq{b}_{ch}")
                    nc.scalar.activation(sq[:], ps[:], AF.Sqrt,
                                         bias=epst[:, 0:1], scale=1.0 / C)
                    nc.vector.reciprocal_approx_fast(
                        invx[:, ch * 512:(ch + 1) * 512], sq[:])
                for ct in range(TC):
                    nc.vector.tensor_mul(xs[ct][:].bitcast(f32r), xt[ct][:],
                                         invx[:])

                # ---------- depthwise band on hs -> hm ----------
                hm = []
                for ct in range(TC):
                    t = sb.tile([128, F], f32, tag=f"hm{ct}", name=f"hm{b}_{ct}")
                    nc.vector.tensor_scalar_mul(t[:].bitcast(f32r),
                                                hs[ct][:, 1:F + 1],
                                                w3t[ct][:, 1:2])
                    nc.vector.scalar_tensor_tensor(
                        t[:].bitcast(f32r), hs[ct][:, 0:F], w3t[ct][:, 0:1],
                        t[:], OP.mult, OP.add)
                    nc.vector.scalar_tensor_tensor(
                        t[:].bitcast(f32r), hs[ct][:, 2:F + 2], w3t[ct][:, 2:3],
                        t[:], OP.mult, OP.add)
                    hm.append(t)

                # ---------- gates + output, per 512-pixel chunk ----------
                for ch in range(NCH):
                    S = slice(ch * 512, ch * 512 + 512)
                    rg, ug, cg = [], [], []
                    for m in range(8):
                        ps = pp.tile([128, 512], f32, tag="gate", bufs=4,
                                     name=f"gps{b}_{ch}_{m}")
                        for k in range(TC):
                            nc.tensor.matmul(
                                ps[:],
                                xw_s[k][:, m * 128:(m + 1) * 128].bitcast(f32r),
                                xs[k][:, S].bitcast(f32r),
                                start=(k == 0), stop=False)
                        for k in range(TC):
                            nc.tensor.matmul(
                                ps[:],
                                hw_s[k][:, m * 128:(m + 1) * 128].bitcast(f32r),
                                hm[k][:, S].bitcast(f32r),
                                start=False, stop=(k == TC - 1))
                        if m < 4:
                            g = sb.tile([128, 512], f32, tag=f"r{m}",
                                        name=f"r{b}_{ch}_{m}")
                            rg.append(g)
                        else:
                            g = sb.tile([128, 512], f32, tag=f"u{m - 4}",
                                        name=f"u{b}_{ch}_{m - 4}")
                            ug.append(g)
                        nc.scalar.activation(g[:], ps[:], AF.Sigmoid,
                                             bias=gbt[:, m:m + 1])
                    for j in range(4):
                        m = 8 + j
                        psx = pp.tile([128, 512], f32, tag="gate", bufs=4,
                                      name=f"npsx{b}_{ch}_{j}")
                        for k in range(TC):
                            nc.tensor.matmul(
                                psx[:],
                                xw_s[k][:, m * 128:(m + 1) * 128].bitcast(f32r),
                                xs[k][:, S].bitcast(f32r),
                                start=(k == 0), stop=(k == TC - 1))
                        psh = pp.tile([128, 512], f32, tag="gate", bufs=4,
                                      name=f"npsh{b}_{ch}_{j}")
                        for k in range(TC):
                            nc.tensor.matmul(
                                psh[:],
                                hw_s[k][:, m * 128:(m + 1) * 128].bitcast(f32r),
                                hm[k][:, S].bitcast(f32r),
                                start=(k == 0), stop=(k == TC - 1))
                        t = sb.tile([128, 512], f32, tag="nscr", bufs=2,
                                    name=f"nt{b}_{ch}_{j}")
                        # t = (hg_n + bh_n) * reset
                        nc.vector.scalar_tensor_tensor(
                            t[:], psh[:], bhnt[:, j:j + 1], rg[j][:],
                            OP.add, OP.mult)
                        nc.vector.tensor_add(t[:], t[:], psx[:])
                        g = sb.tile([128, 512], f32, tag=f"c{j}",
                                    name=f"c{b}_{ch}_{j}")
                        nc.scalar.activation(g[:], t[:], AF.Tanh,
                                             bias=xbnt[:, j:j + 1])
                        cg.append(g)
                    # y = cand + update*(h - cand) + x ; out = y*inv_y*out_w
                    py = pp.tile([128, 512], f32, tag="ynrm", bufs=2,
                                 name=f"yps{b}_{ch}")
                    yt = []
                    for ct in range(TC):
                        s = sb.tile([128, 512], f32, tag="ys", bufs=2,
                                    name=f"ysc{b}_{ch}_{ct}")
                        nc.vector.tensor_sub(s[:], ht[ct][:, S], cg[ct][:])
                        nc.vector.tensor_mul(s[:], s[:], ug[ct][:])
                        nc.vector.tensor_add(s[:], s[:], cg[ct][:])
                        y = sb.tile([128, 512], f32, tag=f"yt{ct}",
                                    name=f"yt{b}_{ch}_{ct}")
                        nc.vector.tensor_add(y[:], s[:], xt[ct][:, S])
                        yt.append(y)
                        y2 = sb.tile([128, 512], f32, tag="y2", bufs=2,
                                     name=f"y2{b}_{ch}_{ct}")
                        nc.scalar.square(y2[:].bitcast(f32r), y[:])
                        nc.tensor.matmul(py[:], onr, y2[:].bitcast(f32r),
                                         start=(ct == 0), stop=(ct == TC - 1))
                    sq = sb.tile([128, 512], f32, tag="sq", bufs=3,
                                 name=f"ysq{b}_{ch}")
                    nc.scalar.activation(sq[:], py[:], AF.Sqrt,
                                         bias=epst[:, 0:1], scale=1.0 / C)
                    iy = sb.tile([128, 512], f32, tag="invy", bufs=2,
                                 name=f"invy{b}_{ch}")
                    nc.vector.reciprocal_approx_fast(iy[:], sq[:])
                    for ct in range(TC):
                        o = sb.tile([128, 512], f32, tag="ot", bufs=3,
                                    name=f"ot{b}_{ch}_{ct}")
                        nc.vector.scalar_tensor_tensor(
                            o[:], yt[ct][:], wont[:, ct:ct + 1], iy[:],
                            OP.mult, OP.mult)
                        nc.sync.dma_start(
                            outd[b, ct * 128:(ct + 1) * 128, S], o[:])

    nc.compile()
    return nc


def _get_program():
    if "nc" not in _CACHE:
        _CACHE["nc"] = _build_program()
    return _CACHE["nc"]


def kernel(x_t, h_prev, in_norm_w, hid_norm_w, out_norm_w,
           xW, xb, hmixW, hmixb, hW, hb):
    from concourse.bass_utils import run_bass_kernel_spmd

    nc = _get_program()

    f = np.float32
    x = np.ascontiguousarray(np.asarray(x_t, f).reshape(B, C, F))
    h = np.ascontiguousarray(np.asarray(h_prev, f).reshape(B, C, F))
    xW = np.asarray(xW, f)
    hW = np.asarray(hW, f)
    xWT = np.ascontiguousarray((xW * np.asarray(in_norm_w, f)[None, :]).T)
    hWT = np.ascontiguousarray(hW.T)
    w3 = np.ascontiguousarray(
        np.asarray(hmixW, f)[:, 0, 0, :] * np.asarray(hid_norm_w, f)[:, None])
    bh = hW @ np.asarray(hmixb, f) + np.asarray(hb, f)
    gb = np.ascontiguousarray((np.asarray(xb, f) + bh).reshape(3 * C, 1))
    bhn = np.ascontiguousarray(bh[2 * C:].reshape(C, 1))
    xbn = np.ascontiguousarray(np.asarray(xb, f)[2 * C:].reshape(C, 1))
    won = np.ascontiguousarray(np.asarray(out_norm_w, f).reshape(C, 1))

    shared = {"xWT": xWT, "hWT": hWT, "w3": w3, "gb": gb, "bhn": bhn,
              "xbn": xbn, "won": won,
              "ones_in": np.ones((128, 128), dtype=f)}
    in_maps = []
    for c in range(N_CORES):
        m = dict(shared)
        m["x"] = x[c * BPC:(c + 1) * BPC]
        m["h"] = h[c * BPC:(c + 1) * BPC]
        in_maps.append(m)

    res = run_bass_kernel_spmd(nc, in_maps, core_ids=list(range(N_CORES)),
                               **_CACHE.get("run_kwargs", {}))
    _CACHE["last_results"] = res
    out = np.concatenate([res.results[c]["out"] for c in range(N_CORES)], axis=0)
    return out.reshape(B, C, 1, F)
